# revision 47
# baseline (speedup 1.0000x reference)
"""Distributed Trainium2 kernel for nn_Attention (B=2, N=2048, C=1024, H=16, HD=64).

Sharding: tensor-parallel over heads.  Core c owns batch b=c//4 and heads
[4*(c%4), 4*(c%4)+4) over the FULL sequence.  Each core computes q/k/v for
its heads (RoPE on q,k), dense softmax attention, and its partial
projection; partials are summed with four pipelined bf16 ReduceScatters
(one per 512-row block) so each core ends with disjoint row slices of the
final output.  No AllGathers are needed at all.

Attention is computed with scores transposed (s^T = k^T q -> [keys, q]) so
exp output feeds the AV matmul as lhsT directly; AV is non-transposed
(out [q, hd+1]) with a ones-column in v producing softmax denominators.
The attention output is transposed for the projection on the PE (identity
matmul; the DMA-xbar transpose would serialize with collectives).

Engine budget (cost model: PE 0.42ns/row, ACT 0.83, DVE 1.04 with 2x for
all-bf16 packed ops, Pool 0.83): PE ~140us is the floor; exp (131k rows)
runs mostly on ACT with five front-phase chunks offloaded to Pool as
pow(e, s) from a DVE-staged SBUF copy (pow is ACT/Pool-only on V3 ISA,
and Pool cannot read PSUM); proj0 is woven into the qc0-AV phase so RS0
dispatches a phase earlier and the RS chain ends ~2us after proj3 lands.  k-RoPE and q-RoPE both run fully in bf16
(psum -> bf16 copy first, swapped-sin table) on Pool and DVE.

Collective discipline (the hard-won part): collectives are gpsimd-only
and, once dispatched, block every later Pool-queue instruction until
they complete (~21.5us each: 15us constant + bytes/40GBps), while also
serializing among themselves on a single device.  Hence: Pool pow only
in phases whose pows complete before RS0 dispatches; RS output staging
never on a queue that later holds compute-gating work (j<2 as direct
DRAM->DRAM on Pool inside the RS chain's input-gaps, j>=2 SBUF-staged on
ACT after its exp stream ends).  Weights/tables load via ACT-hwdge/Pool-
swdge and xT as half-rows on SP so the first matmul chains are gated by
the DMA device, not issue rate.

Emission is generator-based, but the tile scheduler re-derives per-engine
order from dependencies with its own timing model — only dependency/
resource-structure changes (buffer depths, engine assignment, instruction
granularity) move the final schedule.
"""

import sys

if "/opt/trn_rl_repo" not in sys.path:
    sys.path.insert(0, "/opt/trn_rl_repo")

import numpy as np

B, N, C = 2, 2048, 1024
H, HD = 16, 64
NCORES = 8
GB = 4            # tensor-parallel group size (cores per batch)
HPC = H // GB     # 4 heads per core
SC = HD ** -0.5   # attention scale
NQC = N // 512    # 4 query chunks of 512
NKC = N // 128    # 16 key chunks of 128


def build():
    import concourse.bass as bass
    import concourse.mybir as mybir
    import concourse.tile as tile
    from concourse import bacc
    from contextlib import ExitStack

    f32 = mybir.dt.float32
    f32r = mybir.dt.float32r
    bf16 = mybir.dt.bfloat16
    AF = mybir.ActivationFunctionType

    nc = bacc.Bacc(None, target_bir_lowering=False, num_devices=NCORES)

    # ---- per-core external inputs (host pre-shards / pre-transposes) ----
    xT = nc.declare_dram_parameter("xT", [C, N], bf16, isOutput=False)
    wqT = nc.declare_dram_parameter("wqT", [C, 256], bf16, isOutput=False)
    wkT = nc.declare_dram_parameter("wkT", [C, 256], bf16, isOutput=False)
    wvT = nc.declare_dram_parameter("wvT", [C, 256], bf16, isOutput=False)
    wpT = nc.declare_dram_parameter("wpT", [256, C], bf16, isOutput=False)
    cos2 = nc.declare_dram_parameter("cos2", [128, N], bf16, isOutput=False)
    sins2 = nc.declare_dram_parameter("sins2", [128, N], bf16, isOutput=False)
    # partition-swapped signed sin (32<->
    # 32 within each 64-block) for the Pool k-rope: SBUF*SBUF ops must use
    # equal base partitions on real hardware
    sinsk2 = nc.declare_dram_parameter("sinsk2", [128, N], bf16, isOutput=False)
    out = nc.declare_dram_parameter("out", [512, C], bf16, isOutput=True)

    groups = [list(range(GB)), list(range(GB, 2 * GB))]
    mm = nc.tensor.matmul

    with tile.TileContext(nc) as tc:
        with ExitStack() as stack:
            ep = stack.enter_context
            ep(nc.allow_low_precision(reason="bf16 attention within 2e-2 gate"))
            dramp = ep(tc.tile_pool(name="dram", bufs=1, space="DRAM"))
            constp = ep(tc.tile_pool(name="const", bufs=1))
            xtp = ep(tc.tile_pool(name="xTp", bufs=1))
            wp_ = ep(tc.tile_pool(name="wts", bufs=1))
            qkp = ep(tc.tile_pool(name="qk", bufs=1))
            vp = ep(tc.tile_pool(name="vsb", bufs=1))
            ptp = ep(tc.tile_pool(name="pT", bufs=18))
            ropep = ep(tc.tile_pool(name="ropet", bufs=4))
            attnp = ep(tc.tile_pool(name="attn", bufs=1))
            attnTp = ep(tc.tile_pool(name="attnT", bufs=1))
            outp = ep(tc.tile_pool(name="outsb", bufs=2))
            smallp = ep(tc.tile_pool(name="small", bufs=8))
            ps_mm = ep(tc.tile_pool(name="ps_mm", bufs=2, space="PSUM"))
            ps_s = ep(tc.tile_pool(name="ps_s", bufs=2, space="PSUM"))
            ps_av = ep(tc.tile_pool(name="ps_av", bufs=2, space="PSUM"))

            # separate DRAM staging tile per RS block: avoids false
            # (tensor-granularity) deps between RS_j reads and proj_{j+1}
            # writes.  Collectives may not touch external tensors on the
            # PJRT dispatch path, so RS outputs land in internal tiles and
            # are staged to `out` through SBUF.
            rs_in = [
                dramp.tile([512, C], bf16, name=f"rs_in{j}") for j in range(4)
            ]
            rs_out = [
                dramp.tile([128, C], bf16, name=f"rs_out{j}") for j in range(4)
            ]

            # ---- persistent SBUF tiles ----
            cos_sb = constp.tile([128, N], bf16, name="cos_sb")
            sin_sb = constp.tile([128, N], bf16, name="sin_sb")
            sink_sb = constp.tile([128, N], bf16, name="sink_sb")
            # base-e constant for exp-as-pow on Pool (reads the DVE-staged
            # bf16 SBUF copy of the score psum)
            e_sb = constp.tile([128, 2, 512], bf16, name="e_sb")
            nc.gpsimd.memset(e_sb[:, :, :], 2.718281828459045)
            wk_sb = wp_.tile([128, 8, 256], bf16, name="wk_sb")
            wq_sb = wp_.tile([128, 8, 256], bf16, name="wq_sb")
            # issue rate (not device bandwidth) gates the first matmul
            # chains: weights go through the idle ACT hwdge queue (632ns per
            # issue vs ~1us on Pool swdge, and Pool SEQ stays free for the
            # k-rope); the small rope tables ride the Pool swdge
            for cc in range(8):
                nc.scalar.dma_start(wk_sb[:, cc, :], wkT[cc * 128:(cc + 1) * 128, :])
            nc.gpsimd.dma_start(sink_sb[:, :], sinsk2[:, :])
            nc.gpsimd.dma_start(cos_sb[:, :], cos2[:, :])
            nc.gpsimd.dma_start(sin_sb[:, :], sins2[:, :])
            for cc in range(8):
                nc.scalar.dma_start(wq_sb[:, cc, :], wqT[cc * 128:(cc + 1) * 128, :])

            # identity (bf16) for PE transposes; DMA-xbar transposes would
            # serialize with the collectives, so transpose on PE instead
            id_sb = constp.tile([128, 128], bf16, name="id_sb")
            nc.gpsimd.memset(id_sb[:, :], 1.0)
            nc.gpsimd.affine_select(
                id_sb[:, :], id_sb[:, :], pattern=[[1, 128]],
                compare_op=mybir.AluOpType.is_equal, fill=0.0,
                base=0, channel_multiplier=-1,
            )

            xT_sb = xtp.tile([128, 8, N], bf16, name="xT_sb")
            # quarter-row transfers, nc-major: the first k/q chains need
            # all 8 cc of ONE 512-column block, so landing nc0 first
            # (~2.9us of device time) starts the pipeline several us
            # earlier than half- or full-row transfers
            for q4 in range(4):
                for cc in range(8):
                    nc.sync.dma_start(
                        xT_sb[:, cc, q4 * 512:(q4 + 1) * 512],
                        xT[cc * 128:(cc + 1) * 128,
                           q4 * 512:(q4 + 1) * 512],
                    )
            wv_sb = wp_.tile([128, 8, 256], bf16, name="wv_sb")
            for cc in range(8):
                nc.sync.dma_start(wv_sb[:, cc, :], wvT[cc * 128:(cc + 1) * 128, :])
            wp_sb = wp_.tile([128, 2, C], bf16, name="wp_sb")
            for ch in range(2):
                nc.sync.dma_start(wp_sb[:, ch, :], wpT[ch * 128:(ch + 1) * 128, :])

            qT_sb = qkp.tile([128, 2, N], bf16, name="qT_sb")
            kT_sb = qkp.tile([128, 2, N], bf16, name="kT_sb")
            v_sb = vp.tile([128, NKC, HPC, HD + 1], bf16, name="v_sb")
            # softmax-denominator ones column, set once
            nc.gpsimd.memset(v_sb[:, :, :, HD:HD + 1], 1.0)

            attn_sb = attnp.tile([128, 16, HPC, HD], bf16, name="attn_sb")
            attnT_sb = attnTp.tile([128, 2, N], bf16, name="attnT_sb")

            def rope_chunk(eng, src_t, dst, n0, swapped_sin=False):
                """dst = src*cos + rot32(src)*signed_sin, all [128, 512].

                swapped_sin=True uses the partition-pre-swapped sin table so
                both SBUF operands share a base partition (hw constraint);
                use it when src_t lives in SBUF."""
                tdt = bf16 if swapped_sin else f32
                tmp = ropep.tile([128, 512], tdt, name="tmp", tag="ropetmp")
                for lo in (0, 64):
                    if swapped_sin:
                        eng.tensor_mul(
                            tmp[lo:lo + 32, :], src_t[lo + 32:lo + 64, :],
                            sink_sb[lo + 32:lo + 64, n0:n0 + 512],
                        )
                        eng.tensor_mul(
                            tmp[lo + 32:lo + 64, :], src_t[lo:lo + 32, :],
                            sink_sb[lo:lo + 32, n0:n0 + 512],
                        )
                    else:
                        eng.tensor_mul(
                            tmp[lo:lo + 32, :], src_t[lo + 32:lo + 64, :],
                            sin_sb[lo:lo + 32, n0:n0 + 512],
                        )
                        eng.tensor_mul(
                            tmp[lo + 32:lo + 64, :], src_t[lo:lo + 32, :],
                            sin_sb[lo + 32:lo + 64, n0:n0 + 512],
                        )
                eng.tensor_mul(dst, src_t, cos_sb[:, n0:n0 + 512])
                eng.tensor_add(dst, dst, tmp[:, :])

            def kq_gen(hp):
                # 8 chunks, k/q interleaved per n-chunk so the first scores
                # chunk (needs only k-nc0 + q-nc0 roped) can start ~10us
                # earlier.  k-rope on Pool via a DVE PSUM->SBUF copy (Pool
                # cannot read PSUM on real hardware); q-rope on DVE.
                for nc_ in range(NQC):
                    for which, wsb, dstT in (
                        ("k", wk_sb, kT_sb),
                        ("q", wq_sb, qT_sb),
                    ):
                        ps = ps_mm.tile([128, 512], f32, name="ps", tag="mm")
                        for cc in range(8):
                            mm(ps[:, :], wsb[:, cc, hp * 128:(hp + 1) * 128],
                               xT_sb[:, cc, nc_ * 512:(nc_ + 1) * 512],
                               start=cc == 0, stop=cc == 7)
                        dst = dstT[:, hp, nc_ * 512:(nc_ + 1) * 512]
                        # copy psum->sbuf bf16 first: k-rope runs on Pool
                        # (cannot read PSUM on hw), q-rope on DVE gets the
                        # 2x_1p all-bf16 fast path; both use the
                        # partition-swapped sin table (SBUF src)
                        sb = ropep.tile([128, 512], bf16, name="ksb",
                                        tag="ksb", bufs=4)
                        nc.vector.tensor_copy(sb[:, :], ps[:, :])
                        rope_chunk(nc.gpsimd if which == "k" else nc.vector,
                                   sb[:, :], dst, nc_ * 512, swapped_sin=True)
                        yield

            def v_gen():
                # 16 chunks: v[keys, dv] in bf16 (+ones col); copies on DVE
                for kc in range(NKC):
                    ps = ps_mm.tile([128, 512], f32, name="ps", tag="mm")
                    for cc in range(8):
                        mm(ps[:, 0:256], xT_sb[:, cc, kc * 128:(kc + 1) * 128],
                           wv_sb[:, cc, :], start=cc == 0, stop=cc == 7)
                    nc.vector.tensor_copy(
                        v_sb[:, kc, :, 0:HD],
                        ps[:, 0:256].rearrange("p (h d) -> p h d", d=HD),
                    )
                    yield

            pt_tiles = {}

            def scores_gen(hp, qc, route="aaaaaaaa"):
                # 8 chunks (sub, g): 4 score mms + 2 exp instrs -> pT bf16.
                # route[i] picks the exp engine per chunk: 'a'=ACT Exp,
                # 'p'=Pool pow(e,.), 'd'=DVE pow(e,.)  (scores pre-scaled)
                for sub in range(2):
                    h = 2 * hp + sub
                    lo = sub * 64
                    q_ap = qT_sb[lo:lo + 64, hp, qc * 512:(qc + 1) * 512]
                    for g in range(4):
                        eng = route[sub * 4 + g]
                        pt = ptp.tile([128, 4, 512], bf16, name="pt", tag="pt")
                        pt_tiles[(h, qc, g)] = pt
                        for pr in range(2):
                            ps = ps_s.tile([128, 2, 512], f32, name="ps2",
                                           tag="sc")
                            for u in range(2):
                                kc = 4 * g + 2 * pr + u
                                mm(ps[:, u, :],
                                   kT_sb[lo:lo + 64, hp, kc * 128:(kc + 1) * 128],
                                   q_ap, start=True, stop=True)
                            dst = pt[:, 2 * pr:2 * pr + 2, :]
                            if eng == "a":
                                nc.scalar.activation(dst, ps[:, :, :], AF.Exp)
                            else:
                                # Pool pow must read SBUF (no PSUM on real
                                # hw, and DMA cannot read PSUM either):
                                # stage with a DVE copy.  Net: moves 996ns
                                # of exp off ACT for 596ns of DVE + free
                                # Pool time
                                sstg = ropep.tile([128, 2, 512], bf16,
                                                  name="sstg", tag="sstg",
                                                  bufs=8)
                                nc.vector.tensor_copy(sstg[:, :, :],
                                                      ps[:, :, :])
                                nc.gpsimd.tensor_tensor(dst, e_sb[:, :, :],
                                                        sstg[:, :, :],
                                                        mybir.AluOpType.pow)
                        yield

            def av_gen(hp, qc, sub_major=False):
                # 16 chunks, qp-major so each row-pair's attn completes for
                # all heads before the next pair (lets proj weave in early).
                # sub_major=True orders by head instead: the first 8 chunks
                # then depend only on the first head's exp, letting the last
                # phase's AV weave under the tail of its own exp stream.
                # Norms are emitted BEFORE the final yield of each (qp, sub)
                # so no work hides after the last yield.
                outer = range(2)
                for o_ in outer:
                    for i_ in range(2):
                        qp, sub = (i_, o_) if sub_major else (o_, i_)
                        h = 2 * hp + sub
                        pts = [pt_tiles[(h, qc, g)] for g in range(4)]
                        av = ps_av.tile([128, 2, HD + 1], f32, name="av",
                                        tag="av")
                        recip = smallp.tile([128, 2, 1], f32,
                                            name="recip", tag="rc")
                        for i in range(2):
                            qs = qp * 2 + i
                            for gp in range(2):
                                for g in (2 * gp, 2 * gp + 1):
                                    for u in range(4):
                                        kc = 4 * g + u
                                        mm(av[:, i, :],
                                           pts[g][:, u,
                                                  qs * 128:(qs + 1) * 128],
                                           v_sb[:, kc, h, :],
                                           start=kc == 0, stop=kc == NKC - 1)
                                if gp == 1:
                                    qb = qc * 4 + qs
                                    nc.vector.reciprocal(recip[:, i, :],
                                                         av[:, i, HD:HD + 1])
                                    nc.vector.tensor_scalar_mul(
                                        attn_sb[:, qb, h, :], av[:, i, 0:HD],
                                        recip[:, i, :]
                                    )
                                yield

            def proj_gen(qc):
                # 4 chunks (per qb): PE transpose + proj matmuls + copy + DMA
                for qs in range(4):
                    qb = qc * 4 + qs
                    # [128, 2, 130] bf16 = 520B matches the av tag byte size,
                    # so transposes share the av PSUM slots
                    tr = ps_av.tile([128, 2, 130], bf16, name="tr", tag="av")
                    for chh in range(2):
                        mm(tr[:, chh, 0:128],
                           attn_sb[:, qb, 2 * chh:2 * chh + 2, :],
                           id_sb[:, :], is_transpose=True,
                           start=True, stop=True)
                    nc.vector.tensor_copy(
                        attnT_sb[:, :, qb * 128:(qb + 1) * 128], tr[:, :, 0:128]
                    )
                    osb = outp.tile([128, C], bf16, name="osb", tag="osb")
                    for nn in range(2):
                        ps = ps_mm.tile([128, 512], f32, name="ps", tag="mm")
                        for chh in range(2):
                            mm(ps[:, :],
                               attnT_sb[:, chh, qb * 128:(qb + 1) * 128],
                               wp_sb[:, chh, nn * 512:(nn + 1) * 512],
                               start=chh == 0, stop=chh == 1)
                        nc.vector.tensor_copy(osb[:, nn * 512:(nn + 1) * 512],
                                              ps[:, :])
                    nc.sync.dma_start(rs_in[qc][qs * 128:(qs + 1) * 128, :],
                                      osb[:, :])
                    yield

            def emit_rs(j):
                rs = nc.gpsimd.collective_compute(
                    "ReduceScatter", mybir.AluOpType.add,
                    replica_groups=groups,
                    ins=[rs_in[j][:, :].opt()],
                    outs=[rs_out[j][:, :].opt()],
                )
                # a dispatched collective blocks every later Pool-queue
                # instruction until it completes; deprioritize so ready pows
                # are ordered ahead of it by the tile scheduler
                try:
                    rs.ins.bass_priority = (1 << 24) + j
                except AttributeError:
                    pass

            def emit_out(j):
                # RS_j-gated output staging.  Queue choice is delicate: on SP
                # the scheduler orders these ahead of later proj staging (its
                # pass underestimates collective latency) and the RS-wait
                # then stalls compute; so j=0,1 ride Pool (their windows fall
                # in the RS chain's input-gaps) and j=2,3 ride ACT, which has
                # finished its exp stream by then.  ACT uses SBUF staging
                # (DRAM->DRAM runs ~6.3us vs 2x0.8us staged).
                if j < 2:
                    d = nc.gpsimd.dma_start(out[j * 128:(j + 1) * 128, :],
                                            rs_out[j][:, :])
                    ds = (d,)
                else:
                    osb = outp.tile([128, C], bf16, name="osb2", tag="osb2",
                                    bufs=2)
                    d1 = nc.scalar.dma_start(osb[:, :], rs_out[j][:, :])
                    d2 = nc.scalar.dma_start(out[j * 128:(j + 1) * 128, :],
                                             osb[:, :])
                    ds = (d1, d2)
                for d in ds:
                    try:
                        d.ins.bass_priority = (1 << 25) + j
                    except AttributeError:
                        pass

            def weave(slots, *pairs):
                for _ in range(slots):
                    for gen, cnt in pairs:
                        for _ in range(cnt):
                            next(gen, None)

            def drain(gen):
                for _ in gen:
                    pass

            # ---- schedule: phases p0..p7 = s(0,0) s(1,0) s(0,1) s(1,1)
            #      s(0,2) s(1,2) s(0,3) s(1,3); AV of phase k weaves under
            #      phase k+2; V under p1; kq(1) under p0; proj(qc)+RS under
            #      the phase after av(1,qc) completes ----
            g = {}
            RF = "aaaaaaaa"  # front: Pool busy with k-rope
            R = "aapaaapp"   # back: 3/8 chunks to Pool pow
            # weave s00's first chunks between kq0's n-chunks: scores for
            # key-block g only need k/q of n-chunk g roped, so the exp
            # stream starts ~9us earlier instead of waiting for the xT
            # second halves that gate kq0's nc2/nc3 chains
            g["kq0"] = kq_gen(0)
            g["s00"] = scores_gen(0, 0, RF)
            weave(4, (g["kq0"], 2), (g["s00"], 1))
            drain(g["kq0"])
            g["kq1"] = kq_gen(1)
            weave(4, (g["s00"], 1), (g["kq1"], 2))
            drain(g["s00"]), drain(g["kq1"])
            g["s10"], g["v"] = scores_gen(1, 0, "aapaaaap"), v_gen()
            weave(8, (g["s10"], 1), (g["v"], 2))
            drain(g["s10"]), drain(g["v"])
            # both QC0 AV phases under p2 so proj0/RS0 fire one phase
            # earlier — the serialized RS chain is the tail-critical path
            # proj0 chases av(·,0)'s qp-groups inside p2 so RS0 dispatches
            # a phase earlier; p3 then runs all-ACT (its Pool pows would sit
            # inside the advanced RS0 window)
            g["s01"], g["a00"], g["a10"], g["p0"] = (
                scores_gen(0, 1, R), av_gen(0, 0), av_gen(1, 0), proj_gen(0))
            weave(4, (g["s01"], 1), (g["a00"], 2), (g["a10"], 2))
            weave(4, (g["s01"], 1), (g["a00"], 2), (g["a10"], 2),
                  (g["p0"], 1))
            drain(g["s01"]), drain(g["a00"]), drain(g["a10"]), drain(g["p0"])
            emit_rs(0)
            g["s11"], g["a01"] = scores_gen(1, 1, RF), av_gen(0, 1)
            weave(8, (g["s11"], 1), (g["a01"], 2))
            drain(g["s11"]), drain(g["a01"])
            g["s02"], g["a11"], g["p1"] = (
                scores_gen(0, 2, RF), av_gen(1, 1), proj_gen(1))
            weave(4, (g["s02"], 1), (g["a11"], 4))
            drain(g["a11"])
            weave(4, (g["s02"], 1), (g["p1"], 1))
            drain(g["s02"]), drain(g["p1"])
            emit_rs(1)
            g["s12"], g["a02"] = scores_gen(1, 2, "aaappapp"), av_gen(0, 2)
            weave(8, (g["s12"], 1), (g["a02"], 2))
            drain(g["s12"]), drain(g["a02"])
            g["s03"], g["a12"], g["p2"] = (
                scores_gen(0, 3, "ppaaaaaa"), av_gen(1, 2), proj_gen(2))
            # proj2 chunks chase av12's qp-groups (qp0 done after 8 av
            # yields) so block-2 staging — the RS2 dispatch gate — lands
            # ~5us earlier
            weave(2, (g["s03"], 1), (g["a12"], 4))
            weave(2, (g["s03"], 1), (g["a12"], 4), (g["p2"], 1))
            drain(g["a12"])
            weave(4, (g["s03"], 1), (g["p2"], 1))
            drain(g["s03"]), drain(g["p2"])
            emit_rs(2)
            g["s13"], g["a03"] = scores_gen(1, 3, RF), av_gen(0, 3)
            g["a13"], g["p3"] = av_gen(1, 3, sub_major=True), proj_gen(3)
            weave(4, (g["s13"], 1), (g["a03"], 2))
            # av(1,3) head-0 chunks depend only on the first half of the
            # last exp phase — weave them under its tail
            weave(4, (g["s13"], 1), (g["a03"], 2), (g["a13"], 2))
            drain(g["s13"]), drain(g["a03"])
            # drain: head-1 AV row-pairs then their proj row-pairs
            weave(1, (g["a13"], 4), (g["p3"], 2))
            weave(1, (g["a13"], 4), (g["p3"], 2))
            drain(g["a13"]), drain(g["p3"])
            emit_rs(3)
            for j in range(4):
                emit_out(j)

    nc.compile()
    return nc


_NC_CACHE = {}


def _get_nc():
    if "nc" not in _NC_CACHE:
        _NC_CACHE["nc"] = build()
    return _NC_CACHE["nc"]


def make_in_maps(x, cos, sin, qkv_w, proj_w, proj_b):
    import ml_dtypes

    bf16 = ml_dtypes.bfloat16
    x = np.asarray(x, np.float32)
    cos = np.asarray(cos, np.float32)
    sin = np.asarray(sin, np.float32)
    qkv_w = np.asarray(qkv_w, np.float32)
    proj_w = np.asarray(proj_w, np.float32)

    sign = np.concatenate([-np.ones(32, np.float32), np.ones(32, np.float32)])
    cosT = cos.T                                       # [HD, N]
    sinsT = (sin * sign).T                             # [HD, N] signed
    cos2v = np.ascontiguousarray(np.concatenate([cosT, cosT], 0)).astype(bf16)
    sins2v = np.ascontiguousarray(np.concatenate([sinsT, sinsT], 0)).astype(bf16)
    sinsk2v = np.ascontiguousarray(
        sins2v.reshape(2, 2, 32, N)[:, ::-1].reshape(128, N))

    in_maps = []
    for c in range(NCORES):
        b, hg = c // GB, c % GB
        cs = slice(256 * hg, 256 * hg + 256)
        in_maps.append(
            {
                "xT": np.ascontiguousarray(x[b].T).astype(bf16),
                # attention scale folded into wq so scores arrive pre-scaled
                # (lets exp run as pow(e, .) on Pool/DVE with no scale op)
                "wqT": np.ascontiguousarray(
                    qkv_w[0 * C:1 * C][cs].T * SC).astype(bf16),
                "wkT": np.ascontiguousarray(qkv_w[1 * C:2 * C][cs].T).astype(bf16),
                "wvT": np.ascontiguousarray(qkv_w[2 * C:3 * C][cs].T).astype(bf16),
                "wpT": np.ascontiguousarray(proj_w[:, cs].T).astype(bf16),
                "cos2": cos2v,
                "sins2": sins2v,
                "sinsk2": sinsk2v,
            }
        )
    return in_maps


def assemble(results, proj_b):
    out = np.empty((B, N, C), np.float32)
    for c in range(NCORES):
        b, r = c // GB, c % GB
        o = np.asarray(results[c]["out"]).astype(np.float32)
        for j in range(4):
            out[b, 512 * j + 128 * r: 512 * j + 128 * r + 128] = (
                o[128 * j:128 * (j + 1)]
            )
    return out + np.asarray(proj_b, np.float32)


def kernel(x, cos, sin, qkv_w, proj_w, proj_b):
    from concourse.bass_utils import run_bass_kernel_spmd

    nc = _get_nc()
    in_maps = make_in_maps(x, cos, sin, qkv_w, proj_w, proj_b)
    res = run_bass_kernel_spmd(nc, in_maps, core_ids=list(range(NCORES)))
    return assemble(res.results, proj_b)



# revision 48
# speedup vs baseline: 1.0922x; 1.0922x over previous
"""Distributed Trainium2 kernel for nn_Attention (B=2, N=2048, C=1024, H=16, HD=64).

Sharding: tensor-parallel over heads.  Core c owns batch b=c//4 and heads
[4*(c%4), 4*(c%4)+4) over the FULL sequence.  Each core computes q/k/v for
its heads (RoPE on q,k), dense softmax attention, and its partial
projection; partials are summed with four pipelined bf16 ReduceScatters
(one per 512-row block) so each core ends with disjoint row slices of the
final output.  No AllGathers are needed at all.

Attention is computed with scores transposed (s^T = k^T q -> [keys, q]) so
exp output feeds the AV matmul as lhsT directly; AV is non-transposed
(out [q, hd+1]) with a ones-column in v producing softmax denominators.
The attention output is transposed for the projection on the PE (identity
matmul; the DMA-xbar transpose would serialize with collectives).

Engine budget (cost model: PE 0.42ns/row, ACT 0.83, DVE 1.04 with 2x for
all-bf16 packed ops, Pool 0.83): PE ~140us is the floor; exp (131k rows)
runs mostly on ACT with five front-phase chunks offloaded to Pool as
pow(e, s) from a DVE-staged SBUF copy (pow is ACT/Pool-only on V3 ISA,
and Pool cannot read PSUM); proj0 is woven into the qc0-AV phase so RS0
dispatches a phase earlier and the RS chain ends ~2us after proj3 lands.  k-RoPE and q-RoPE both run fully in bf16
(psum -> bf16 copy first, swapped-sin table) on Pool and DVE.

Collective discipline (the hard-won part): collectives are gpsimd-only
and, once dispatched, block every later Pool-queue instruction until
they complete (~21.5us each: 15us constant + bytes/40GBps), while also
serializing among themselves on a single device.  Hence: Pool pow only
in phases whose pows complete before RS0 dispatches; RS output staging
never on a queue that later holds compute-gating work (j<2 as direct
DRAM->DRAM on Pool inside the RS chain's input-gaps, j>=2 SBUF-staged on
ACT after its exp stream ends).  Weights/tables load via ACT-hwdge/Pool-
swdge and xT as half-rows on SP so the first matmul chains are gated by
the DMA device, not issue rate.

Emission is generator-based, but the tile scheduler re-derives per-engine
order from dependencies with its own timing model — only dependency/
resource-structure changes (buffer depths, engine assignment, instruction
granularity) move the final schedule.
"""

import sys

if "/opt/trn_rl_repo" not in sys.path:
    sys.path.insert(0, "/opt/trn_rl_repo")

import numpy as np

B, N, C = 2, 2048, 1024
H, HD = 16, 64
NCORES = 8
GB = 4            # tensor-parallel group size (cores per batch)
HPC = H // GB     # 4 heads per core
SC = HD ** -0.5   # attention scale
NQC = N // 512    # 4 query chunks of 512
NKC = N // 128    # 16 key chunks of 128


def build():
    import concourse.bass as bass
    import concourse.mybir as mybir
    import concourse.tile as tile
    from concourse import bacc
    from contextlib import ExitStack

    f32 = mybir.dt.float32
    f32r = mybir.dt.float32r
    bf16 = mybir.dt.bfloat16
    AF = mybir.ActivationFunctionType

    nc = bacc.Bacc(None, target_bir_lowering=False, num_devices=NCORES)

    # ---- per-core external inputs (host pre-shards / pre-transposes) ----
    xT = nc.declare_dram_parameter("xT", [C, N], bf16, isOutput=False)
    wqT = nc.declare_dram_parameter("wqT", [C, 256], bf16, isOutput=False)
    wkT = nc.declare_dram_parameter("wkT", [C, 256], bf16, isOutput=False)
    wvT = nc.declare_dram_parameter("wvT", [C, 256], bf16, isOutput=False)
    wpT = nc.declare_dram_parameter("wpT", [256, C], bf16, isOutput=False)
    cos2 = nc.declare_dram_parameter("cos2", [128, N], bf16, isOutput=False)
    sins2 = nc.declare_dram_parameter("sins2", [128, N], bf16, isOutput=False)
    # partition-swapped signed sin (32<->
    # 32 within each 64-block) for the Pool k-rope: SBUF*SBUF ops must use
    # equal base partitions on real hardware
    sinsk2 = nc.declare_dram_parameter("sinsk2", [128, N], bf16, isOutput=False)
    out = nc.declare_dram_parameter("out", [512, C], bf16, isOutput=True)

    groups = [list(range(GB)), list(range(GB, 2 * GB))]
    mm = nc.tensor.matmul

    with tile.TileContext(nc) as tc:
        with ExitStack() as stack:
            ep = stack.enter_context
            ep(nc.allow_low_precision(reason="bf16 attention within 2e-2 gate"))
            dramp = ep(tc.tile_pool(name="dram", bufs=1, space="DRAM"))
            constp = ep(tc.tile_pool(name="const", bufs=1))
            xtp = ep(tc.tile_pool(name="xTp", bufs=1))
            wp_ = ep(tc.tile_pool(name="wts", bufs=1))
            qkp = ep(tc.tile_pool(name="qk", bufs=1))
            vp = ep(tc.tile_pool(name="vsb", bufs=1))
            ptp = ep(tc.tile_pool(name="pT", bufs=18))
            ropep = ep(tc.tile_pool(name="ropet", bufs=4))
            attnp = ep(tc.tile_pool(name="attn", bufs=1))
            attnTp = ep(tc.tile_pool(name="attnT", bufs=1))
            outp = ep(tc.tile_pool(name="outsb", bufs=2))
            smallp = ep(tc.tile_pool(name="small", bufs=8))
            ps_mm = ep(tc.tile_pool(name="ps_mm", bufs=2, space="PSUM"))
            ps_s = ep(tc.tile_pool(name="ps_s", bufs=2, space="PSUM"))
            ps_av = ep(tc.tile_pool(name="ps_av", bufs=2, space="PSUM"))

            # separate DRAM staging tile per RS block: avoids false
            # (tensor-granularity) deps between RS_j reads and proj_{j+1}
            # writes.  Collectives may not touch external tensors on the
            # PJRT dispatch path, so RS outputs land in internal tiles and
            # are staged to `out` through SBUF.
            rs_in = [
                dramp.tile([512, C], bf16, name=f"rs_in{j}") for j in range(4)
            ]
            rs_out = [
                dramp.tile([128, C], bf16, name=f"rs_out{j}") for j in range(4)
            ]

            # ---- persistent SBUF tiles ----
            cos_sb = constp.tile([128, N], bf16, name="cos_sb")
            sin_sb = constp.tile([128, N], bf16, name="sin_sb")
            sink_sb = constp.tile([128, N], bf16, name="sink_sb")
            # base-e constant for exp-as-pow on Pool (reads the DVE-staged
            # bf16 SBUF copy of the score psum)
            e_sb = constp.tile([128, 2, 512], bf16, name="e_sb")
            nc.gpsimd.memset(e_sb[:, :, :], 2.718281828459045)
            wk_sb = wp_.tile([128, 8, 256], bf16, name="wk_sb")
            wq_sb = wp_.tile([128, 8, 256], bf16, name="wq_sb")
            # issue rate (not device bandwidth) gates the first matmul
            # chains: weights go through the idle ACT hwdge queue (632ns per
            # issue vs ~1us on Pool swdge, and Pool SEQ stays free for the
            # k-rope); the small rope tables ride the Pool swdge
            for cc in range(8):
                nc.scalar.dma_start(wk_sb[:, cc, :], wkT[cc * 128:(cc + 1) * 128, :])
            nc.gpsimd.dma_start(sink_sb[:, :], sinsk2[:, :])
            nc.gpsimd.dma_start(cos_sb[:, :], cos2[:, :])
            nc.gpsimd.dma_start(sin_sb[:, :], sins2[:, :])
            for cc in range(8):
                nc.scalar.dma_start(wq_sb[:, cc, :], wqT[cc * 128:(cc + 1) * 128, :])

            # identity (bf16) for PE transposes; DMA-xbar transposes would
            # serialize with the collectives, so transpose on PE instead
            id_sb = constp.tile([128, 128], bf16, name="id_sb")
            nc.gpsimd.memset(id_sb[:, :], 1.0)
            nc.gpsimd.affine_select(
                id_sb[:, :], id_sb[:, :], pattern=[[1, 128]],
                compare_op=mybir.AluOpType.is_equal, fill=0.0,
                base=0, channel_multiplier=-1,
            )

            xT_sb = xtp.tile([128, 8, N], bf16, name="xT_sb")
            # quarter-row transfers, nc-major: the first k/q chains need
            # all 8 cc of ONE 512-column block, so landing nc0 first
            # (~2.9us of device time) starts the pipeline several us
            # earlier than half- or full-row transfers
            for q4 in range(4):
                for cc in range(8):
                    nc.sync.dma_start(
                        xT_sb[:, cc, q4 * 512:(q4 + 1) * 512],
                        xT[cc * 128:(cc + 1) * 128,
                           q4 * 512:(q4 + 1) * 512],
                    )
            wv_sb = wp_.tile([128, 8, 256], bf16, name="wv_sb")
            for cc in range(8):
                nc.sync.dma_start(wv_sb[:, cc, :], wvT[cc * 128:(cc + 1) * 128, :])
            wp_sb = wp_.tile([128, 2, C], bf16, name="wp_sb")
            for ch in range(2):
                nc.sync.dma_start(wp_sb[:, ch, :], wpT[ch * 128:(ch + 1) * 128, :])

            qT_sb = qkp.tile([128, 2, N], bf16, name="qT_sb")
            kT_sb = qkp.tile([128, 2, N], bf16, name="kT_sb")
            v_sb = vp.tile([128, NKC, HPC, HD + 1], bf16, name="v_sb")
            # softmax-denominator ones column, set once
            nc.gpsimd.memset(v_sb[:, :, :, HD:HD + 1], 1.0)

            attn_sb = attnp.tile([128, 16, HPC, HD], bf16, name="attn_sb")
            attnT_sb = attnTp.tile([128, 2, N], bf16, name="attnT_sb")

            def rope_chunk(eng, src_t, dst, n0, swapped_sin=False):
                """dst = src*cos + rot32(src)*signed_sin, all [128, 512].

                swapped_sin=True uses the partition-pre-swapped sin table so
                both SBUF operands share a base partition (hw constraint);
                use it when src_t lives in SBUF."""
                tdt = bf16 if swapped_sin else f32
                tmp = ropep.tile([128, 512], tdt, name="tmp", tag="ropetmp")
                for lo in (0, 64):
                    if swapped_sin:
                        eng.tensor_mul(
                            tmp[lo:lo + 32, :], src_t[lo + 32:lo + 64, :],
                            sink_sb[lo + 32:lo + 64, n0:n0 + 512],
                        )
                        eng.tensor_mul(
                            tmp[lo + 32:lo + 64, :], src_t[lo:lo + 32, :],
                            sink_sb[lo:lo + 32, n0:n0 + 512],
                        )
                    else:
                        eng.tensor_mul(
                            tmp[lo:lo + 32, :], src_t[lo + 32:lo + 64, :],
                            sin_sb[lo:lo + 32, n0:n0 + 512],
                        )
                        eng.tensor_mul(
                            tmp[lo + 32:lo + 64, :], src_t[lo:lo + 32, :],
                            sin_sb[lo + 32:lo + 64, n0:n0 + 512],
                        )
                eng.tensor_mul(dst, src_t, cos_sb[:, n0:n0 + 512])
                eng.tensor_add(dst, dst, tmp[:, :])

            def kq_gen(hp):
                # 8 chunks, k/q interleaved per n-chunk so the first scores
                # chunk (needs only k-nc0 + q-nc0 roped) can start ~10us
                # earlier.  k-rope on Pool via a DVE PSUM->SBUF copy (Pool
                # cannot read PSUM on real hardware); q-rope on DVE.
                for nc_ in range(NQC):
                    for which, wsb, dstT in (
                        ("k", wk_sb, kT_sb),
                        ("q", wq_sb, qT_sb),
                    ):
                        ps = ps_mm.tile([128, 512], f32, name="ps", tag="mm")
                        for cc in range(8):
                            mm(ps[:, :], wsb[:, cc, hp * 128:(hp + 1) * 128],
                               xT_sb[:, cc, nc_ * 512:(nc_ + 1) * 512],
                               start=cc == 0, stop=cc == 7)
                        dst = dstT[:, hp, nc_ * 512:(nc_ + 1) * 512]
                        # copy psum->sbuf bf16 first: k-rope runs on Pool
                        # (cannot read PSUM on hw), q-rope on DVE gets the
                        # 2x_1p all-bf16 fast path; both use the
                        # partition-swapped sin table (SBUF src)
                        sb = ropep.tile([128, 512], bf16, name="ksb",
                                        tag="ksb", bufs=4)
                        nc.vector.tensor_copy(sb[:, :], ps[:, :])
                        rope_chunk(nc.gpsimd if which == "k" else nc.vector,
                                   sb[:, :], dst, nc_ * 512, swapped_sin=True)
                        yield

            def v_gen():
                # 16 chunks: v[keys, dv] in bf16 (+ones col); copies on DVE
                for kc in range(NKC):
                    ps = ps_mm.tile([128, 512], f32, name="ps", tag="mm")
                    for cc in range(8):
                        mm(ps[:, 0:256], xT_sb[:, cc, kc * 128:(kc + 1) * 128],
                           wv_sb[:, cc, :], start=cc == 0, stop=cc == 7)
                    nc.vector.tensor_copy(
                        v_sb[:, kc, :, 0:HD],
                        ps[:, 0:256].rearrange("p (h d) -> p h d", d=HD),
                    )
                    yield

            pt_tiles = {}

            def scores_gen(hp, qc, route="aaaaaaaa"):
                # 8 chunks (sub, g): 4 score mms + 2 exp instrs -> pT bf16.
                # route[i] picks the exp engine per chunk: 'a'=ACT Exp,
                # 'p'=Pool pow(e,.), 'd'=DVE pow(e,.)  (scores pre-scaled)
                for sub in range(2):
                    h = 2 * hp + sub
                    lo = sub * 64
                    q_ap = qT_sb[lo:lo + 64, hp, qc * 512:(qc + 1) * 512]
                    for g in range(4):
                        eng = route[sub * 4 + g]
                        pt = ptp.tile([128, 4, 512], bf16, name="pt", tag="pt")
                        pt_tiles[(h, qc, g)] = pt
                        for pr in range(2):
                            ps = ps_s.tile([128, 2, 512], f32, name="ps2",
                                           tag="sc")
                            for u in range(2):
                                kc = 4 * g + 2 * pr + u
                                mm(ps[:, u, :],
                                   kT_sb[lo:lo + 64, hp, kc * 128:(kc + 1) * 128],
                                   q_ap, start=True, stop=True)
                            dst = pt[:, 2 * pr:2 * pr + 2, :]
                            if eng == "a":
                                nc.scalar.activation(dst, ps[:, :, :], AF.Exp)
                            else:
                                # Pool pow must read SBUF (no PSUM on real
                                # hw, and DMA cannot read PSUM either):
                                # stage with a DVE copy.  Net: moves 996ns
                                # of exp off ACT for 596ns of DVE + free
                                # Pool time
                                sstg = ropep.tile([128, 2, 512], bf16,
                                                  name="sstg", tag="sstg",
                                                  bufs=8)
                                nc.vector.tensor_copy(sstg[:, :, :],
                                                      ps[:, :, :])
                                nc.gpsimd.tensor_tensor(dst, e_sb[:, :, :],
                                                        sstg[:, :, :],
                                                        mybir.AluOpType.pow)
                        yield

            def av_gen(hp, qc, sub_major=False):
                # 16 chunks, qp-major so each row-pair's attn completes for
                # all heads before the next pair (lets proj weave in early).
                # sub_major=True orders by head instead: the first 8 chunks
                # then depend only on the first head's exp, letting the last
                # phase's AV weave under the tail of its own exp stream.
                # Norms are emitted BEFORE the final yield of each (qp, sub)
                # so no work hides after the last yield.
                outer = range(2)
                for o_ in outer:
                    for i_ in range(2):
                        qp, sub = (i_, o_) if sub_major else (o_, i_)
                        h = 2 * hp + sub
                        pts = [pt_tiles[(h, qc, g)] for g in range(4)]
                        av = ps_av.tile([128, 2, HD + 1], f32, name="av",
                                        tag="av")
                        recip = smallp.tile([128, 2, 1], f32,
                                            name="recip", tag="rc")
                        for i in range(2):
                            qs = qp * 2 + i
                            for gp in range(2):
                                for g in (2 * gp, 2 * gp + 1):
                                    for u in range(4):
                                        kc = 4 * g + u
                                        mm(av[:, i, :],
                                           pts[g][:, u,
                                                  qs * 128:(qs + 1) * 128],
                                           v_sb[:, kc, h, :],
                                           start=kc == 0, stop=kc == NKC - 1)
                                if gp == 1:
                                    qb = qc * 4 + qs
                                    nc.vector.reciprocal(recip[:, i, :],
                                                         av[:, i, HD:HD + 1])
                                    nc.vector.tensor_scalar_mul(
                                        attn_sb[:, qb, h, :], av[:, i, 0:HD],
                                        recip[:, i, :]
                                    )
                                yield

            def proj_gen(qc):
                # 4 chunks (per qb): PE transpose + proj matmuls + copy + DMA
                for qs in range(4):
                    qb = qc * 4 + qs
                    # [128, 2, 130] bf16 = 520B matches the av tag byte size,
                    # so transposes share the av PSUM slots
                    tr = ps_av.tile([128, 2, 130], bf16, name="tr", tag="av")
                    for chh in range(2):
                        mm(tr[:, chh, 0:128],
                           attn_sb[:, qb, 2 * chh:2 * chh + 2, :],
                           id_sb[:, :], is_transpose=True,
                           start=True, stop=True)
                    nc.vector.tensor_copy(
                        attnT_sb[:, :, qb * 128:(qb + 1) * 128], tr[:, :, 0:128]
                    )
                    osb = outp.tile([128, C], bf16, name="osb", tag="osb")
                    for nn in range(2):
                        ps = ps_mm.tile([128, 512], f32, name="ps", tag="mm")
                        for chh in range(2):
                            mm(ps[:, :],
                               attnT_sb[:, chh, qb * 128:(qb + 1) * 128],
                               wp_sb[:, chh, nn * 512:(nn + 1) * 512],
                               start=chh == 0, stop=chh == 1)
                        nc.vector.tensor_copy(osb[:, nn * 512:(nn + 1) * 512],
                                              ps[:, :])
                    nc.sync.dma_start(rs_in[qc][qs * 128:(qs + 1) * 128, :],
                                      osb[:, :])
                    yield

            def emit_rs(j):
                rs = nc.gpsimd.collective_compute(
                    "ReduceScatter", mybir.AluOpType.add,
                    replica_groups=groups,
                    ins=[rs_in[j][:, :].opt()],
                    outs=[rs_out[j][:, :].opt()],
                )
                # a dispatched collective blocks every later Pool-queue
                # instruction until it completes; deprioritize so ready pows
                # are ordered ahead of it by the tile scheduler
                try:
                    rs.ins.bass_priority = (1 << 24) + j
                except AttributeError:
                    pass

            def emit_out(j):
                # RS_j-gated output staging.  Queue choice is delicate: on SP
                # the scheduler orders these ahead of later proj staging (its
                # pass underestimates collective latency) and the RS-wait
                # then stalls compute; so j=0,1 ride Pool (their windows fall
                # in the RS chain's input-gaps) and j=2,3 ride ACT, which has
                # finished its exp stream by then.  ACT uses SBUF staging
                # (DRAM->DRAM runs ~6.3us vs 2x0.8us staged).
                if j < 2:
                    d = nc.gpsimd.dma_start(out[j * 128:(j + 1) * 128, :],
                                            rs_out[j][:, :])
                    ds = (d,)
                else:
                    osb = outp.tile([128, C], bf16, name="osb2", tag="osb2",
                                    bufs=2)
                    d1 = nc.scalar.dma_start(osb[:, :], rs_out[j][:, :])
                    d2 = nc.scalar.dma_start(out[j * 128:(j + 1) * 128, :],
                                             osb[:, :])
                    ds = (d1, d2)
                for d in ds:
                    try:
                        d.ins.bass_priority = (1 << 25) + j
                    except AttributeError:
                        pass

            def weave(slots, *pairs):
                for _ in range(slots):
                    for gen, cnt in pairs:
                        for _ in range(cnt):
                            next(gen, None)

            def drain(gen):
                for _ in gen:
                    pass

            # ---- schedule: phases p0..p7 = s(0,0) s(1,0) s(0,1) s(1,1)
            #      s(0,2) s(1,2) s(0,3) s(1,3); AV of phase k weaves under
            #      phase k+2; V under p1; kq(1) under p0; proj(qc)+RS under
            #      the phase after av(1,qc) completes ----
            g = {}
            RF = "aaaaaaaa"  # front: Pool busy with k-rope
            R = "aapaaapp"   # back: 3/8 chunks to Pool pow
            # weave s00's first chunks between kq0's n-chunks: scores for
            # key-block g only need k/q of n-chunk g roped, so the exp
            # stream starts ~9us earlier instead of waiting for the xT
            # second halves that gate kq0's nc2/nc3 chains
            g["kq0"] = kq_gen(0)
            g["s00"] = scores_gen(0, 0, RF)
            weave(4, (g["kq0"], 2), (g["s00"], 1))
            drain(g["kq0"])
            g["kq1"] = kq_gen(1)
            weave(4, (g["s00"], 1), (g["kq1"], 2))
            drain(g["s00"]), drain(g["kq1"])
            g["s10"], g["v"] = scores_gen(1, 0, "aapaaaap"), v_gen()
            weave(8, (g["s10"], 1), (g["v"], 2))
            drain(g["s10"]), drain(g["v"])
            # both QC0 AV phases under p2 so proj0/RS0 fire one phase
            # earlier — the serialized RS chain is the tail-critical path
            # proj0 chases av(·,0)'s qp-groups inside p2 so RS0 dispatches
            # a phase earlier; p3 then runs all-ACT (its Pool pows would sit
            # inside the advanced RS0 window)
            g["s01"], g["a00"], g["a10"], g["p0"] = (
                scores_gen(0, 1, R), av_gen(0, 0), av_gen(1, 0), proj_gen(0))
            weave(4, (g["s01"], 1), (g["a00"], 2), (g["a10"], 2))
            weave(4, (g["s01"], 1), (g["a00"], 2), (g["a10"], 2),
                  (g["p0"], 1))
            drain(g["s01"]), drain(g["a00"]), drain(g["a10"]), drain(g["p0"])
            emit_rs(0)
            g["s11"], g["a01"] = scores_gen(1, 1, RF), av_gen(0, 1)
            weave(8, (g["s11"], 1), (g["a01"], 2))
            drain(g["s11"]), drain(g["a01"])
            g["s02"], g["a11"], g["p1"] = (
                scores_gen(0, 2, RF), av_gen(1, 1), proj_gen(1))
            weave(4, (g["s02"], 1), (g["a11"], 4))
            drain(g["a11"])
            weave(4, (g["s02"], 1), (g["p1"], 1))
            drain(g["s02"]), drain(g["p1"])
            emit_rs(1)
            g["s12"], g["a02"] = scores_gen(1, 2, RF), av_gen(0, 2)
            weave(8, (g["s12"], 1), (g["a02"], 2))
            drain(g["s12"]), drain(g["a02"])
            g["s03"], g["a12"], g["p2"] = (
                scores_gen(0, 3, RF), av_gen(1, 2), proj_gen(2))
            # proj2 chunks chase av12's qp-groups (qp0 done after 8 av
            # yields) so block-2 staging — the RS2 dispatch gate — lands
            # ~5us earlier
            weave(2, (g["s03"], 1), (g["a12"], 4))
            weave(2, (g["s03"], 1), (g["a12"], 4), (g["p2"], 1))
            drain(g["a12"])
            weave(4, (g["s03"], 1), (g["p2"], 1))
            drain(g["s03"]), drain(g["p2"])
            emit_rs(2)
            g["s13"], g["a03"] = scores_gen(1, 3, RF), av_gen(0, 3)
            g["a13"], g["p3"] = av_gen(1, 3, sub_major=True), proj_gen(3)
            weave(4, (g["s13"], 1), (g["a03"], 2))
            # av(1,3) head-0 chunks depend only on the first half of the
            # last exp phase — weave them under its tail
            weave(4, (g["s13"], 1), (g["a03"], 2), (g["a13"], 2))
            drain(g["s13"]), drain(g["a03"])
            # drain: head-1 AV row-pairs then their proj row-pairs
            weave(1, (g["a13"], 4), (g["p3"], 2))
            weave(1, (g["a13"], 4), (g["p3"], 2))
            drain(g["a13"]), drain(g["p3"])
            emit_rs(3)
            for j in range(4):
                emit_out(j)

    nc.compile()
    return nc


_NC_CACHE = {}


def _get_nc():
    if "nc" not in _NC_CACHE:
        _NC_CACHE["nc"] = build()
    return _NC_CACHE["nc"]


def make_in_maps(x, cos, sin, qkv_w, proj_w, proj_b):
    import ml_dtypes

    bf16 = ml_dtypes.bfloat16
    x = np.asarray(x, np.float32)
    cos = np.asarray(cos, np.float32)
    sin = np.asarray(sin, np.float32)
    qkv_w = np.asarray(qkv_w, np.float32)
    proj_w = np.asarray(proj_w, np.float32)

    sign = np.concatenate([-np.ones(32, np.float32), np.ones(32, np.float32)])
    cosT = cos.T                                       # [HD, N]
    sinsT = (sin * sign).T                             # [HD, N] signed
    cos2v = np.ascontiguousarray(np.concatenate([cosT, cosT], 0)).astype(bf16)
    sins2v = np.ascontiguousarray(np.concatenate([sinsT, sinsT], 0)).astype(bf16)
    sinsk2v = np.ascontiguousarray(
        sins2v.reshape(2, 2, 32, N)[:, ::-1].reshape(128, N))

    in_maps = []
    for c in range(NCORES):
        b, hg = c // GB, c % GB
        cs = slice(256 * hg, 256 * hg + 256)
        in_maps.append(
            {
                "xT": np.ascontiguousarray(x[b].T).astype(bf16),
                # attention scale folded into wq so scores arrive pre-scaled
                # (lets exp run as pow(e, .) on Pool/DVE with no scale op)
                "wqT": np.ascontiguousarray(
                    qkv_w[0 * C:1 * C][cs].T * SC).astype(bf16),
                "wkT": np.ascontiguousarray(qkv_w[1 * C:2 * C][cs].T).astype(bf16),
                "wvT": np.ascontiguousarray(qkv_w[2 * C:3 * C][cs].T).astype(bf16),
                "wpT": np.ascontiguousarray(proj_w[:, cs].T).astype(bf16),
                "cos2": cos2v,
                "sins2": sins2v,
                "sinsk2": sinsk2v,
            }
        )
    return in_maps


def assemble(results, proj_b):
    out = np.empty((B, N, C), np.float32)
    for c in range(NCORES):
        b, r = c // GB, c % GB
        o = np.asarray(results[c]["out"]).astype(np.float32)
        for j in range(4):
            out[b, 512 * j + 128 * r: 512 * j + 128 * r + 128] = (
                o[128 * j:128 * (j + 1)]
            )
    return out + np.asarray(proj_b, np.float32)


def kernel(x, cos, sin, qkv_w, proj_w, proj_b):
    from concourse.bass_utils import run_bass_kernel_spmd

    nc = _get_nc()
    in_maps = make_in_maps(x, cos, sin, qkv_w, proj_w, proj_b)
    res = run_bass_kernel_spmd(nc, in_maps, core_ids=list(range(NCORES)))
    return assemble(res.results, proj_b)

